# revision 14
# baseline (speedup 1.0000x reference)
"""Trainium2 Bass kernel for CompanySpecificHeads (MoE-style routed MLP heads), v2.

Semantics (matching the reference):
    out[b] = gelu(z[b] @ W1[cid[b]] + b1[cid[b]]) @ W2[cid[b]] + b2[cid[b]]

Expert-parallel across 8 NeuronCores, 8 companies per core. Key changes vs v1:

  * W1 is streamed as float8 E3M4 (4 mantissa bits) with a fixed power-of-2
    prescale folded out in the gelu activation's scale. This halves the
    dominant HBM traffic (8MB -> 4MB per core) and halves LDWEIGHTS time
    (FWL reads 4 fp8/cycle vs 2 fp16). End-to-end rel err ~1.3e-2 < 2e-2.
    The moving operand (tokens) stays fp16 - mixed-dtype matmul is allowed.
  * Exact per-slot token widths instead of a global padded capacity:
    companies are sorted by token count into 8 slots of 8 (one company per
    core per slot); slot width = max count in slot, padded to 4. All cores
    share the same widths (SPMD single program); padding waste ~4%.
  * Bias via a K=4 selector matmul (psum has_written must be set by the PE),
    pre-scaled by SCALE so gelu(psum/SCALE) is exact.
  * L2 (w2 dot) stays on the PE, software-pipelined one company behind L1
    so the PE never waits on the ACT engine's gelu.
  * DMA: w1 per company on the sync HWDGE ring (first company first);
    consts + tail z on the scalar ring; first-two-slot z on gpsimd after the
    warmup memset. Output staged in SBUF, one HWDGE store at the end.
"""

import numpy as np

B, C, D, H = 4096, 64, 512, 1024
NCORES = 8
CPC = C // NCORES
KC = D // 128      # contraction chunks of 128
HC = H // 128      # h chunks of 128
SCALE = 16.0       # W1 prescale before e3m4 quantization
# PE warmup: HAM un-throttles (1.2->2.4GHz) only after ~3.4us of sustained
# fp16-path matmul activity, and the e3m4 L1 matmuls do NOT register as
# activity (measured: 27us of dense e3m4 matmuls never flipped the clock).
# So burn a full window with dense fp16 warmup matmuls; the fp16 bias and
# L2 matmuls every ~1us keep it warm afterwards.
WARMUP = 12
WARMW = 512

_COMPILED = {}


def _build(widths):
    """Build the Bass/Tile program for per-slot token widths `widths`."""
    import concourse.bass as bass
    import concourse.bacc as bacc
    import concourse.mybir as mybir
    from concourse.tile import TileContext
    from contextlib import ExitStack

    f32 = mybir.dt.float32
    f16 = mybir.dt.float16
    f8e3 = mybir.dt.float8e3

    NSLOT = len(widths)
    Wmax = max(widths)
    SELW = KC * Wmax
    B1W = NSLOT * 2 * 128
    cum = np.concatenate([[0], np.cumsum(widths)])
    NTOT = int(cum[-1])

    gelu = mybir.ActivationFunctionType.Gelu

    nc = bacc.Bacc(None, target_bir_lowering=False)

    zt_d = nc.dram_tensor("zt", [128, KC * NTOT], f16, kind="ExternalInput")
    w1_d = nc.dram_tensor("w1", [NSLOT, 128, 2 * KC * (H // 2)], f8e3,
                          kind="ExternalInput")
    # full 128 contraction rows (4 data + 124 zero) so the bias matmul runs
    # in the normal full-array mode instead of the slower row_grp=q0 mode
    cst_d = nc.dram_tensor("cst", [128, SELW + B1W], f16, kind="ExternalInput")
    w2_d = nc.dram_tensor("w2", [128, NSLOT * HC], f16, kind="ExternalInput")
    out_d = nc.dram_tensor("out", [1, NTOT], f32, kind="ExternalOutput")

    with TileContext(nc) as tc, ExitStack() as ctx:
        const = ctx.enter_context(tc.tile_pool(name="const", bufs=1))

        # Warmup scratch: memset is gpsimd's first instruction so the PE can
        # start ramping the HAM clock right after engine boot.
        wsc = const.tile([128, WARMW], f16)
        nc.gpsimd.memset(wsc[:], 0.0)

        # Scalar HWDGE ring, in consumption order: first two slots of z,
        # then the small consts, then the remaining z.
        zall = const.tile([128, KC * NTOT], f16)
        z01 = int(KC * cum[min(2, NSLOT)])
        nc.scalar.dma_start(out=zall[:, :z01], in_=zt_d[:, :z01])
        ct = const.tile([128, SELW + B1W], f16)
        nc.scalar.dma_start(out=ct[:], in_=cst_d[:])
        w2t = const.tile([128, NSLOT * HC], f16)
        nc.scalar.dma_start(out=w2t[:], in_=w2_d[:])
        if z01 < KC * NTOT:
            nc.scalar.dma_start(out=zall[:, z01:], in_=zt_d[:, z01:])

        # Staged per-slot outputs; single store at the end.
        oall = const.tile([1, NTOT], f32)

        # Per-company weights on the sync HWDGE ring, one DMA per company,
        # first company first (4KB contiguous per partition, full-rate).
        w1p = ctx.enter_context(tc.tile_pool(name="w1p", bufs=1))
        w1ts = []
        for s in range(NSLOT):
            w1t = w1p.tile([128, 2, KC, H // 2], f8e3, name=f"w1_{s}")
            nc.sync.dma_start(out=w1t[:], in_=w1_d[s])
            w1ts.append(w1t)

        hp = ctx.enter_context(tc.tile_pool(name="hp", bufs=6))
        pp = ctx.enter_context(tc.tile_pool(name="pp", bufs=4, space="PSUM"))
        opp = ctx.enter_context(tc.tile_pool(name="opp", bufs=2, space="PSUM"))
        wps = ctx.enter_context(tc.tile_pool(name="wps", bufs=1, space="PSUM"))

        wp = wps.tile([128, WARMW], f32)
        for _ in range(WARMUP):
            nc.tensor.matmul(wp[:], wsc[:, :128], wsc[:], start=True, stop=True)

        sel = ct[:, 0:SELW].rearrange("p (j t) -> p j t", j=KC)
        b1t = ct[:, SELW:SELW + B1W].rearrange("p (s g m) -> p s g m",
                                               s=NSLOT, g=2)

        def do_l2(s, W, off, hts):
            osum = opp.tile([1, Wmax], f32)
            for g in range(2):
                for j in range(KC):
                    jj = KC * g + j
                    nc.tensor.matmul(
                        osum[:, :W],
                        w2t[:, HC * s + jj:HC * s + jj + 1],
                        hts[g][:, j * W:(j + 1) * W],
                        start=(jj == 0),
                        stop=(jj == HC - 1),
                    )
            nc.vector.tensor_copy(oall[:, off:off + W], osum[:, :W])

        stored = [False]
        prev = None
        for s in range(NSLOT):
            W = widths[s]
            off = int(cum[s])
            zc = zall[:, KC * off:KC * (off + W)].rearrange(
                "p (k t) -> p k t", k=KC)
            w1t = w1ts[s]
            hts = []
            for g in range(2):
                ps = pp.tile([128, KC * Wmax], f32)
                psb = ps[:, 0:KC * W].rearrange("p (j t) -> p j t", j=KC)
                for k in range(KC):
                    for j in range(KC):
                        nc.tensor.matmul(
                            ps[:, j * W:(j + 1) * W],
                            w1t[:, g, k, 128 * j:128 * (j + 1)],
                            zc[:, k, :],
                            start=(k == 0),
                            stop=False,
                        )
                # bias last: ps[m, (j,t)] += SCALE*b1[...,128j+m] via selector
                # (emitted after the L1 chain so company 0's L1 needn't wait
                # for the consts DMA)
                nc.tensor.matmul(psb, b1t[:, s, g, :], sel[:, :, 0:W],
                                 start=False, stop=True)
                ht = hp.tile([128, KC * Wmax], f16)
                nc.scalar.activation(ht[:, 0:KC * W], ps[:, 0:KC * W], gelu,
                                     scale=1.0 / SCALE)
                hts.append(ht)
            if prev is not None:
                do_l2(*prev)
                # early partial store: overlap the store's issue+receipt
                # latency with the last companies' compute
                if prev[0] == NSLOT - 2:
                    so = int(cum[NSLOT - 1])
                    nc.scalar.dma_start(out=out_d[:, :so], in_=oall[:, :so])
                    stored[0] = so
            prev = (s, W, off, hts)
        do_l2(*prev)

        so = stored[0] or 0
        nc.scalar.dma_start(out=out_d[:, so:], in_=oall[:, so:])

    nc.finalize()
    return nc


def _get_compiled(widths):
    key = tuple(widths)
    if key not in _COMPILED:
        _COMPILED[key] = _build(list(widths))
    return _COMPILED[key]


def kernel(z, company_id, W1, b1, W2, b2):
    import ml_dtypes
    from concourse.bass_utils import run_bass_kernel_spmd

    z = np.asarray(z, dtype=np.float32)
    cid = np.asarray(company_id).astype(np.int64).ravel()
    W1 = np.asarray(W1, dtype=np.float32)
    b1 = np.asarray(b1, dtype=np.float32)
    W2 = np.asarray(W2, dtype=np.float32)
    b2 = np.asarray(b2, dtype=np.float32)
    O = W2.shape[2]

    idx_by_company = [np.nonzero(cid == gc)[0] for gc in range(C)]

    # Segment any company with >128 tokens (rare) into <=128-token chunks.
    segs = []  # (gc, tok_start, seg_len)
    for gc in range(C):
        n = len(idx_by_company[gc])
        st = 0
        while st < n or (st == 0 and n == 0):
            ln = min(128, n - st)
            segs.append((gc, st, ln))
            st += max(ln, 1)
            if n == 0:
                break
    # pad to a multiple of NCORES with dummy zero-token segments
    while len(segs) % NCORES != 0:
        segs.append((0, 0, 0))

    # Sort descending; slot k gets segs[8k:8k+8] (one per core); shared width.
    segs.sort(key=lambda t: -t[2])
    NSLOT = len(segs) // NCORES
    widths = []
    for k in range(NSLOT):
        mx = max(t[2] for t in segs[k * NCORES:(k + 1) * NCORES])
        widths.append(max(4, ((mx + 3) // 4) * 4))
    cum = np.concatenate([[0], np.cumsum(widths)])
    NTOT = int(cum[-1])
    Wmax = max(widths)
    SELW = KC * Wmax
    B1W = NSLOT * 2 * 128

    nc = _get_compiled(widths)

    # selector: sel[k, j*Wmax + t] = 1 if j == k
    sel = np.zeros((KC, KC, Wmax), dtype=np.float16)
    for k in range(KC):
        sel[k, k, :] = 1.0
    sel = sel.reshape(KC, SELW)

    in_maps = []
    core_slots = []  # per core: list of (gc, tok_indices) per slot
    for core in range(NCORES):
        slots = [segs[k * NCORES + core] for k in range(NSLOT)]
        core_slots.append(slots)

        zt = np.zeros((128, KC * NTOT), dtype=np.float16)
        w1 = np.zeros((NSLOT, 128, 2 * KC * (H // 2)),
                      dtype=ml_dtypes.float8_e3m4)
        b1h = np.zeros((KC, NSLOT, 2, 128), dtype=np.float16)
        w2h = np.zeros((128, NSLOT * HC), dtype=np.float16)

        for s, (gc, st, ln) in enumerate(slots):
            W = widths[s]
            if ln > 0:
                ix = idx_by_company[gc][st:st + ln]
                # zt block: [128, KC, W]; zt[p, k, t] = z[tok, 128k+p]
                zb = np.zeros((KC, 128, W), dtype=np.float16)
                zb[:, :, :ln] = (
                    z[ix].reshape(ln, KC, 128).transpose(1, 2, 0)
                )
                zt[:, KC * cum[s]:KC * (cum[s] + W)] = (
                    zb.transpose(1, 0, 2).reshape(128, KC * W)
                )
            # w1[s][p][g*KC*512 + k*512 + hh] = SCALE*W1[gc, 128k+p, 512g+hh]
            w1[s] = (
                (W1[gc] * SCALE)
                .reshape(KC, 128, 2, H // 2)
                .transpose(1, 2, 0, 3)
                .reshape(128, 2 * KC * (H // 2))
                .astype(ml_dtypes.float8_e3m4)
            )
            # b1h[k, s, g, m] = SCALE*b1[gc, 512g+128k+m]
            b1h[:, s] = (
                (b1[gc] * SCALE).reshape(2, KC, 128).transpose(1, 0, 2)
            ).astype(np.float16)
            # w2h[p, HC*s + jj] = W2[gc, 128jj+p, 0]
            w2h[:, HC * s:HC * (s + 1)] = (
                W2[gc, :, 0].reshape(HC, 128).T.astype(np.float16)
            )

        cst = np.zeros((128, SELW + B1W), dtype=np.float16)
        cst[:KC, :SELW] = sel
        cst[:KC, SELW:] = b1h.reshape(KC, B1W)
        in_maps.append({
            "zt": np.ascontiguousarray(zt),
            "w1": np.ascontiguousarray(w1),
            "cst": np.ascontiguousarray(cst),
            "w2": np.ascontiguousarray(w2h),
        })

    res = run_bass_kernel_spmd(nc, in_maps, list(range(NCORES)))

    out = np.zeros((B, O), dtype=np.float32)
    for core in range(NCORES):
        core_out = res.results[core]["out"].reshape(-1)
        for s, (gc, st, ln) in enumerate(core_slots[core]):
            if ln == 0:
                continue
            ix = idx_by_company[gc][st:st + ln]
            out[ix, 0] = core_out[cum[s]:cum[s] + ln] + b2[gc, 0]
    return out


# revision 20
# speedup vs baseline: 1.0880x; 1.0880x over previous
"""Trainium2 Bass kernel for CompanySpecificHeads (MoE-style routed MLP heads), v2.

Semantics (matching the reference):
    out[b] = gelu(z[b] @ W1[cid[b]] + b1[cid[b]]) @ W2[cid[b]] + b2[cid[b]]

Expert-parallel across 8 NeuronCores, 8 companies per core. Key changes vs v1:

  * W1 is streamed as float8 E3M4 (4 mantissa bits) with a fixed power-of-2
    prescale folded out in the gelu activation's scale. This halves the
    dominant HBM traffic (8MB -> 4MB per core) and halves LDWEIGHTS time
    (FWL reads 4 fp8/cycle vs 2 fp16). End-to-end rel err ~1.3e-2 < 2e-2.
    The moving operand (tokens) stays fp16 - mixed-dtype matmul is allowed.
  * Exact per-slot token widths instead of a global padded capacity:
    companies are sorted by token count into 8 slots of 8 (one company per
    core per slot); slot width = max count in slot, padded to 4. All cores
    share the same widths (SPMD single program); padding waste ~4%.
  * Bias via a K=4 selector matmul (psum has_written must be set by the PE),
    pre-scaled by SCALE so gelu(psum/SCALE) is exact.
  * L2 (w2 dot) stays on the PE, software-pipelined one company behind L1
    so the PE never waits on the ACT engine's gelu.
  * DMA: w1 per company on the sync HWDGE ring (first company first);
    consts + tail z on the scalar ring; first-two-slot z on gpsimd after the
    warmup memset. Output staged in SBUF, one HWDGE store at the end.
"""

import numpy as np

B, C, D, H = 4096, 64, 512, 1024
NCORES = 8
CPC = C // NCORES
KC = D // 128      # contraction chunks of 128
HC = H // 128      # h chunks of 128
SCALE = 16.0       # W1 prescale before e3m4 quantization
# PE warmup: HAM un-throttles (1.2->2.4GHz) only after ~3.4us of sustained
# fp16-path matmul activity, and the e3m4 L1 matmuls do NOT register as
# activity (measured: 27us of dense e3m4 matmuls never flipped the clock).
# So burn a full window with dense fp16 warmup matmuls; the fp16 bias and
# L2 matmuls every ~1us keep it warm afterwards.
WARMUP = 12
WARMW = 512

_COMPILED = {}


def _build(widths):
    """Build the Bass/Tile program for per-slot token widths `widths`."""
    import concourse.bass as bass
    import concourse.bacc as bacc
    import concourse.mybir as mybir
    from concourse.tile import TileContext
    from contextlib import ExitStack

    f32 = mybir.dt.float32
    f16 = mybir.dt.float16
    f8e3 = mybir.dt.float8e3

    NSLOT = len(widths)
    Wmax = max(widths)
    SELW = KC * Wmax
    B1W = NSLOT * 2 * 128
    cum = np.concatenate([[0], np.cumsum(widths)])
    NTOT = int(cum[-1])

    gelu = mybir.ActivationFunctionType.Gelu

    nc = bacc.Bacc(None, target_bir_lowering=False)

    zt_d = nc.dram_tensor("zt", [128, KC * NTOT], f16, kind="ExternalInput")
    w1_d = nc.dram_tensor("w1", [NSLOT, 128, 2 * KC * (H // 2)], f8e3,
                          kind="ExternalInput")
    # full 128 contraction rows (4 data + 124 zero) so the bias matmul runs
    # in the normal full-array mode instead of the slower row_grp=q0 mode;
    # the b1 zero rows are produced by a gpsimd memset to keep the DMA small
    cst_d = nc.dram_tensor("cst", [128, SELW], f16, kind="ExternalInput")
    b1_d = nc.dram_tensor("b1d", [KC, B1W], f16, kind="ExternalInput")
    w2_d = nc.dram_tensor("w2", [128, NSLOT * HC], f16, kind="ExternalInput")
    out_d = nc.dram_tensor("out", [1, NTOT], f32, kind="ExternalOutput")

    with TileContext(nc) as tc, ExitStack() as ctx:
        const = ctx.enter_context(tc.tile_pool(name="const", bufs=1))

        # Warmup scratch: memset is gpsimd's first instruction so the PE can
        # start ramping the HAM clock right after engine boot.
        wsc = const.tile([128, WARMW], f16)
        nc.gpsimd.memset(wsc[:], 0.0)
        b1pad = const.tile([128, B1W], f16)
        nc.gpsimd.memset(b1pad[:], 0.0)

        # Scalar HWDGE ring, in consumption order: first two slots of z,
        # then the small consts, then the remaining z.
        zall = const.tile([128, KC * NTOT], f16)
        z01 = int(KC * cum[min(2, NSLOT)])
        nc.scalar.dma_start(out=zall[:, :z01], in_=zt_d[:, :z01])
        ct = const.tile([128, SELW], f16)
        nc.scalar.dma_start(out=ct[:], in_=cst_d[:])
        nc.scalar.dma_start(out=b1pad[0:KC, :], in_=b1_d[:])
        w2t = const.tile([128, NSLOT * HC], f16)
        nc.scalar.dma_start(out=w2t[:], in_=w2_d[:])
        if z01 < KC * NTOT:
            nc.scalar.dma_start(out=zall[:, z01:], in_=zt_d[:, z01:])

        # Staged per-slot outputs; single store at the end.
        oall = const.tile([1, NTOT], f32)

        # Per-company weights on the sync HWDGE ring, one DMA per company,
        # first company first (4KB contiguous per partition, full-rate).
        w1p = ctx.enter_context(tc.tile_pool(name="w1p", bufs=1))
        w1ts = []
        for s in range(NSLOT):
            w1t = w1p.tile([128, 2, KC, H // 2], f8e3, name=f"w1_{s}")
            nc.sync.dma_start(out=w1t[:], in_=w1_d[s])
            w1ts.append(w1t)

        hp = ctx.enter_context(tc.tile_pool(name="hp", bufs=6))
        pp = ctx.enter_context(tc.tile_pool(name="pp", bufs=4, space="PSUM"))
        opp = ctx.enter_context(tc.tile_pool(name="opp", bufs=2, space="PSUM"))
        wps = ctx.enter_context(tc.tile_pool(name="wps", bufs=1, space="PSUM"))

        wp = wps.tile([128, WARMW], f32)
        for _ in range(WARMUP):
            nc.tensor.matmul(wp[:], wsc[:, :128], wsc[:], start=True, stop=True)

        sel = ct[:, 0:SELW].rearrange("p (j t) -> p j t", j=KC)
        b1t = b1pad[:].rearrange("p (s g m) -> p s g m", s=NSLOT, g=2)

        def do_l2(s, W, off, hts):
            osum = opp.tile([1, Wmax], f32)
            for g in range(2):
                for j in range(KC):
                    jj = KC * g + j
                    nc.tensor.matmul(
                        osum[:, :W],
                        w2t[:, HC * s + jj:HC * s + jj + 1],
                        hts[g][:, j * W:(j + 1) * W],
                        start=(jj == 0),
                        stop=(jj == HC - 1),
                    )
            nc.vector.tensor_copy(oall[:, off:off + W], osum[:, :W])

        stored = [False]
        prev = None
        for s in range(NSLOT):
            W = widths[s]
            off = int(cum[s])
            zc = zall[:, KC * off:KC * (off + W)].rearrange(
                "p (k t) -> p k t", k=KC)
            w1t = w1ts[s]
            hts = []
            for g in range(2):
                ps = pp.tile([128, KC * Wmax], f32)
                psb = ps[:, 0:KC * W].rearrange("p (j t) -> p j t", j=KC)
                # start=True ONLY on the first matmul: the start flag clears
                # has_written bits for the WHOLE bank, so a second start=True
                # would wipe already-accumulated regions
                for k in range(KC):
                    for j in range(KC):
                        nc.tensor.matmul(
                            ps[:, j * W:(j + 1) * W],
                            w1t[:, g, k, 128 * j:128 * (j + 1)],
                            zc[:, k, :],
                            start=(k == 0 and j == 0),
                            stop=False,
                        )
                # bias last: ps[m, (j,t)] += SCALE*b1[...,128j+m] via selector
                # (emitted after the L1 chain so company 0's L1 needn't wait
                # for the consts DMA)
                nc.tensor.matmul(psb, b1t[:, s, g, :], sel[:, :, 0:W],
                                 start=False, stop=True)
                ht = hp.tile([128, KC * Wmax], f16)
                nc.scalar.activation(ht[:, 0:KC * W], ps[:, 0:KC * W], gelu,
                                     scale=1.0 / SCALE)
                hts.append(ht)
            if prev is not None:
                do_l2(*prev)
                # early partial store: overlap the store's issue+receipt
                # latency with the last companies' compute
                if prev[0] == NSLOT - 2:
                    so = int(cum[NSLOT - 1])
                    nc.scalar.dma_start(out=out_d[:, :so], in_=oall[:, :so])
                    stored[0] = so
            prev = (s, W, off, hts)
        do_l2(*prev)

        so = stored[0] or 0
        nc.scalar.dma_start(out=out_d[:, so:], in_=oall[:, so:])

    nc.finalize()
    return nc


def _get_compiled(widths):
    key = tuple(widths)
    if key not in _COMPILED:
        _COMPILED[key] = _build(list(widths))
    return _COMPILED[key]


def kernel(z, company_id, W1, b1, W2, b2):
    import ml_dtypes
    from concourse.bass_utils import run_bass_kernel_spmd

    z = np.asarray(z, dtype=np.float32)
    cid = np.asarray(company_id).astype(np.int64).ravel()
    W1 = np.asarray(W1, dtype=np.float32)
    b1 = np.asarray(b1, dtype=np.float32)
    W2 = np.asarray(W2, dtype=np.float32)
    b2 = np.asarray(b2, dtype=np.float32)
    O = W2.shape[2]

    idx_by_company = [np.nonzero(cid == gc)[0] for gc in range(C)]

    # Segment any company with >128 tokens (rare) into <=128-token chunks.
    segs = []  # (gc, tok_start, seg_len)
    for gc in range(C):
        n = len(idx_by_company[gc])
        st = 0
        while st < n or (st == 0 and n == 0):
            ln = min(128, n - st)
            segs.append((gc, st, ln))
            st += max(ln, 1)
            if n == 0:
                break
    # pad to a multiple of NCORES with dummy zero-token segments
    while len(segs) % NCORES != 0:
        segs.append((0, 0, 0))

    # Sort descending; slot k gets segs[8k:8k+8] (one per core); shared width.
    segs.sort(key=lambda t: -t[2])
    NSLOT = len(segs) // NCORES
    widths = []
    for k in range(NSLOT):
        mx = max(t[2] for t in segs[k * NCORES:(k + 1) * NCORES])
        widths.append(max(4, ((mx + 3) // 4) * 4))
    cum = np.concatenate([[0], np.cumsum(widths)])
    NTOT = int(cum[-1])
    Wmax = max(widths)
    SELW = KC * Wmax
    B1W = NSLOT * 2 * 128

    nc = _get_compiled(widths)

    # selector: sel[k, j*Wmax + t] = 1 if j == k
    sel = np.zeros((KC, KC, Wmax), dtype=np.float16)
    for k in range(KC):
        sel[k, k, :] = 1.0
    sel = sel.reshape(KC, SELW)

    in_maps = []
    core_slots = []  # per core: list of (gc, tok_indices) per slot
    for core in range(NCORES):
        slots = [segs[k * NCORES + core] for k in range(NSLOT)]
        core_slots.append(slots)

        zt = np.zeros((128, KC * NTOT), dtype=np.float16)
        w1 = np.zeros((NSLOT, 128, 2 * KC * (H // 2)),
                      dtype=ml_dtypes.float8_e3m4)
        b1h = np.zeros((KC, NSLOT, 2, 128), dtype=np.float16)
        w2h = np.zeros((128, NSLOT * HC), dtype=np.float16)

        for s, (gc, st, ln) in enumerate(slots):
            W = widths[s]
            if ln > 0:
                ix = idx_by_company[gc][st:st + ln]
                # zt block: [128, KC, W]; zt[p, k, t] = z[tok, 128k+p]
                zb = np.zeros((KC, 128, W), dtype=np.float16)
                zb[:, :, :ln] = (
                    z[ix].reshape(ln, KC, 128).transpose(1, 2, 0)
                )
                zt[:, KC * cum[s]:KC * (cum[s] + W)] = (
                    zb.transpose(1, 0, 2).reshape(128, KC * W)
                )
            # w1[s][p][g*KC*512 + k*512 + hh] = SCALE*W1[gc, 128k+p, 512g+hh]
            w1[s] = (
                (W1[gc] * SCALE)
                .reshape(KC, 128, 2, H // 2)
                .transpose(1, 2, 0, 3)
                .reshape(128, 2 * KC * (H // 2))
                .astype(ml_dtypes.float8_e3m4)
            )
            # b1h[k, s, g, m] = SCALE*b1[gc, 512g+128k+m]
            b1h[:, s] = (
                (b1[gc] * SCALE).reshape(2, KC, 128).transpose(1, 0, 2)
            ).astype(np.float16)
            # w2h[p, HC*s + jj] = W2[gc, 128jj+p, 0]
            w2h[:, HC * s:HC * (s + 1)] = (
                W2[gc, :, 0].reshape(HC, 128).T.astype(np.float16)
            )

        cst = np.zeros((128, SELW), dtype=np.float16)
        cst[:KC, :] = sel
        in_maps.append({
            "zt": np.ascontiguousarray(zt),
            "w1": np.ascontiguousarray(w1),
            "cst": np.ascontiguousarray(cst),
            "b1d": np.ascontiguousarray(b1h.reshape(KC, B1W)),
            "w2": np.ascontiguousarray(w2h),
        })

    res = run_bass_kernel_spmd(nc, in_maps, list(range(NCORES)))

    out = np.zeros((B, O), dtype=np.float32)
    for core in range(NCORES):
        core_out = res.results[core]["out"].reshape(-1)
        for s, (gc, st, ln) in enumerate(core_slots[core]):
            if ln == 0:
                continue
            ix = idx_by_company[gc][st:st + ln]
            out[ix, 0] = core_out[cum[s]:cum[s] + ln] + b2[gc, 0]
    return out
